# revision 8
# baseline (speedup 1.0000x reference)
"""Trainium2 Bass kernel for nn_HSMSSD (HSM-SSD layer, EfficientViM-style).

Full shapes: x [32, 512, 4096], H=W=64, S=64, d_model=d_inner=512.
Sharding: data-parallel over batch, 4 batches per core on 8 cores.

Math per batch (reference.py):
  bcdt = DWConv3x3(W_bcdt @ x + b_bcdt) + b_dw          [3S, L]
  Bm, Cm, dt = split(bcdt); Asm = softmax(dt + A, -1)   (A cancels: shift-inv)
  AB = Asm * Bm
  h  = x @ AB^T                                          [C, S]
  hz = W_hz @ h + b_hz; h1, z = split(hz)
  hg = h1*silu(z) + h1*Dskip
  ho = W_out @ hg + b_out                                [C, S]
  y  = ho @ Cm -> [C, H, W]
  returns (y, ho)

Device mapping (per core):
  - Spatial grid padded W->W+2 ("W2") with zero columns so the 3x3 conv's
    horizontal taps never wrap across rows. L2 = H*W2 (4224 = 33*128).
  - A1: U = W_bcdt @ x as PE matmuls (bf16 in, fp32 psum).
  - A2: conv as 9 accumulated diagonal matmuls per output chunk, reading U
    at shifted column offsets (vertical edges via partial-width matmuls).
  - softmax: ACT exp with fused per-row accumulate (row sums); the 1/sum is
    folded into h^T as a per-partition scale. Pad cols of dt set to -1e30 so
    exp()=0 there.
  - AB (bf16) transposed 128 cols at a time via DMA-transpose; h^T = AB @ x^T
    accumulated over 33 l-chunks with host-provided x^T (bf16).
  - h^T -> h via PE transpose; hz/gate/ho batched across the core's 4
    batches; y = ho @ Cm in fp32 with K=S=64.
"""

import os
import sys
import time
import math

for _p in ("/opt/trn_rl_repo", "/root/.axon_site/_ro/trn_rl_repo"):
    if os.path.isdir(_p) and _p not in sys.path:
        sys.path.insert(0, _p)

import numpy as np
import ml_dtypes

import concourse.bass as bass
import concourse.bacc as bacc
import concourse.mybir as mybir
import concourse.tile as tile
from concourse.bass import MemorySpace
from concourse.bass_utils import run_bass_kernel_spmd

F32 = mybir.dt.float32
BF16 = mybir.dt.bfloat16
AF = mybir.ActivationFunctionType
ALU = mybir.AluOpType

N_CORES = 8


def _mchunks(n):
    """Split n output rows into <=128-row chunks."""
    out = []
    off = 0
    while off < n:
        w = min(128, n - off)
        out.append((off, w))
        off += w
    return out


def emit_core_program(tc, aps, dims, use_bias, dskip_val):
    """Emit one core's program. aps: dict of DRAM APs. dims: (BC,C,S,H,W)."""
    nc = tc.nc
    BC, C, S, H, W = dims
    W2 = W + 2
    L2 = H * W2
    NW = 512
    nch = (L2 + NW - 1) // NW          # A/I column chunks
    KC = C // 128                      # contraction tiles over channels
    NL = L2 // 128                     # l-chunks for step-6 (L2 % 128 == 0)
    assert L2 % 128 == 0 and C % 128 == 0 and S == 64 and (3 * S) == 192

    # tap list: center first (always full range) so it carries start=True
    taps = [(0, 0)] + [(di, dj) for di in (-1, 0, 1) for dj in (-1, 0, 1)
                       if not (di == 0 and dj == 0)]

    with (tc.tile_pool(name="res", bufs=1) as res,
          tc.tile_pool(name="pA", bufs=4, space="PSUM") as pA,
          tc.tile_pool(name="pS", bufs=3, space="PSUM") as pS,
          tc.tile_pool(name="work", bufs=2) as work,
          tc.tile_pool(name="xpool", bufs=5) as xpool,
          tc.tile_pool(name="xtpool", bufs=6) as xtpool,
          tc.tile_pool(name="cmpool", bufs=BC) as cmpool,
          tc.tile_pool(name="small", bufs=2) as small,
          tc.tile_pool(name="ystage", bufs=4) as ystage):
        # ---- resident weights ----
        w1t_sb = []
        for k in range(KC):
            t = res.tile([128, 3 * S], BF16, tag=f"w1t{k}")
            nc.sync.dma_start(t[:], aps["w1t"][128 * k:128 * (k + 1), :])
            w1t_sb.append(t)
        dg0 = res.tile([128, 9, 128], BF16, tag="dg0")
        nc.sync.dma_start(dg0[:], aps["dg0"].rearrange("t p c -> p t c"))
        dg1 = res.tile([64, 9, 64], BF16, tag="dg1")
        nc.sync.dma_start(dg1[:], aps["dg1"].rearrange("t p c -> p t c"))
        whz_sb, wout_sb = [], []
        for k in range(KC):
            t = res.tile([128, 2 * C], BF16, tag=f"whz{k}")
            nc.sync.dma_start(t[:], aps["whzT"][128 * k:128 * (k + 1), :])
            whz_sb.append(t)
            t2 = res.tile([128, C], BF16, tag=f"wout{k}")
            nc.sync.dma_start(t2[:], aps["woutT"][128 * k:128 * (k + 1), :])
            wout_sb.append(t2)
        idf = res.tile([128, 64], F32, tag="idf")
        nc.sync.dma_start(idf[:], aps["identF"][:])
        if use_bias:
            bb1 = res.tile([128, 2], F32, tag="bb1")
            nc.sync.dma_start(bb1[:, 0:1], aps["b_bcdt"][0:128, :])
            nc.sync.dma_start(bb1[0:64, 1:2], aps["b_bcdt"][128:192, :])
            bdw = res.tile([128, 2], F32, tag="bdw")
            nc.sync.dma_start(bdw[:, 0:1], aps["b_dw"][0:128, :])
            nc.sync.dma_start(bdw[0:64, 1:2], aps["b_dw"][128:192, :])
            bhz = res.tile([128, 2 * C // 128], F32, tag="bhz")
            nc.sync.dma_start(bhz[:], aps["b_hz"].rearrange("(a p) one -> p a one",
                                                            p=128).opt())
            bout_rep = res.tile([64, C], F32, tag="bout_rep")
            nc.sync.dma_start(bout_rep[:], aps["bout_rep"][:])

        hnat = [res.tile([128, BC * S], BF16, tag=f"hnat{k}", name=f"hnat{k}") for k in range(KC)]
        hoT = [res.tile([64, C], F32, tag=f"hoT{b}", name=f"hoT{b}") for b in range(BC)]
        hoTb = [res.tile([64, C], BF16, tag=f"hoTb{b}", name=f"hoTb{b}") for b in range(BC)]
        cm_tiles = []

        def copyer(i):
            # alternate psum->sbuf copy engine
            return nc.vector if (i % 2 == 0) else nc.scalar

        def copy_out(eng_i, dst, src, bias_ap):
            """psum->sbuf copy, optional per-partition bias add, dtype cast."""
            if bias_ap is not None:
                nc.vector.tensor_scalar_add(dst, src, bias_ap)
            elif eng_i % 2 == 0:
                nc.vector.tensor_copy(dst, src)
            else:
                nc.scalar.copy(dst, src)

        eng_rr = 0  # round robin counter for copies

        # ================= per batch: A1/A2/softmax/ABT/h^T =================
        hT = []
        for b in range(BC):
            x_sb = []
            for k in range(KC):
                t = xpool.tile([128, L2], BF16, tag="x")
                nc.sync.dma_start(t[:], aps["xb"][b, 128 * k:128 * (k + 1), :])
                x_sb.append(t)

            # ---- A1: U = W1 @ x (+b_bcdt), pad cols re-zeroed if biased ----
            U0 = work.tile([128, L2], BF16, tag="U0", bufs=1)
            U1 = work.tile([64, L2], BF16, tag="U1", bufs=1)
            for n in range(nch):
                n0 = n * NW
                nw = min(NW, L2 - n0)
                for mi, (moff, mw) in enumerate(((0, 128), (128, 64))):
                    P = pA.tile([128, NW], F32, tag="pA")
                    for k in range(KC):
                        nc.tensor.matmul(
                            P[:mw, :nw],
                            lhsT=w1t_sb[k][:, moff:moff + mw],
                            rhs=x_sb[k][:, n0:n0 + nw],
                            start=(k == 0), stop=(k == KC - 1))
                    dst = U0 if mi == 0 else U1
                    copy_out(eng_rr, dst[:mw, n0:n0 + nw], P[:mw, :nw],
                             (bb1[:mw, mi:mi + 1]) if use_bias else None)
                    eng_rr += 1
            if use_bias:
                # zero the pad columns so conv edge taps read 0, not the bias
                for U in (U0, U1):
                    pads = U[:, :].rearrange("p (h w) -> p h w", w=W2)
                    nc.vector.memset(pads[:, :, 0:W2:W2 - 1], 0.0)

            # ---- A2: conv = sum_tap diag(w_tap) @ U[:, +shift] ----
            Bm = work.tile([64, L2], BF16, tag="Bm", bufs=1)
            Cm = cmpool.tile([64, L2], BF16, tag="Cm")
            dtf = work.tile([64, L2], BF16, tag="dtf", bufs=1)
            for n in range(nch):
                n0 = n * NW
                nw = min(NW, L2 - n0)
                for mi, (moff, mw) in enumerate(((0, 128), (128, 64))):
                    P = pA.tile([128, NW], F32, tag="pA")
                    U = U0 if mi == 0 else U1
                    dg = dg0 if mi == 0 else dg1
                    for ti, (di, dj) in enumerate(taps):
                        sh = W2 * di + dj
                        a = max(0, -(n0 + sh))
                        e = min(nw, L2 - n0 - sh)
                        if e <= a:
                            continue
                        nc.tensor.matmul(
                            P[:mw, a:e],
                            lhsT=dg[:, ti, :mw],
                            rhs=U[:mw, n0 + sh + a:n0 + sh + e],
                            start=(ti == 0), stop=(ti == len(taps) - 1))
                    if mi == 0:
                        copy_out(eng_rr, Bm[:, n0:n0 + nw], P[0:64, :nw],
                                 bdw[0:64, 0:1] if use_bias else None)
                        eng_rr += 1
                        copy_out(eng_rr, Cm[:, n0:n0 + nw], P[64:128, :nw],
                                 bdw[64:128, 0:1] if use_bias else None)
                        eng_rr += 1
                    else:
                        copy_out(eng_rr, dtf[:, n0:n0 + nw], P[0:64, :nw],
                                 bdw[0:64, 1:2] if use_bias else None)
                        eng_rr += 1
            cm_tiles.append(Cm)

            # ---- pads of dt -> -inf so exp()=0 there ----
            dt3 = dtf[:, :].rearrange("p (h w) -> p h w", w=W2)
            nc.vector.memset(dt3[:, :, 0:W2:W2 - 1], -1e30)

            # ---- softmax pieces: E=exp(dt) (bf16) + row sums; AB = E*Bm ----
            E = work.tile([64, L2], BF16, tag="E", bufs=1)
            sums = small.tile([64, 1], F32, tag="sums")
            nc.scalar.activation(E[:], dtf[:], AF.Exp, accum_out=sums[:])
            rsum = small.tile([64, 1], F32, tag="rsum")
            nc.vector.reciprocal(rsum[:], sums[:])
            nc.vector.tensor_mul(E[:], E[:], Bm[:])   # AB, in place, bf16

            # ---- ABT via DMA transpose (bf16, 128-col chunks) ----
            ABT = work.tile([128, NL, 64], BF16, tag="ABT")
            for i in range(NL):
                nc.sync.dma_start(ABT[:, i, :], E[:, 128 * i:128 * (i + 1)],
                                  transpose=True)

            # ---- h^T = AB @ x^T  (accumulate over NL chunks of 128 l's) ----
            Ph = pS.tile([128, NW], F32, tag="pS")
            for i in range(NL):
                xt_sb = xtpool.tile([128, C], BF16, tag="xt")
                nc.sync.dma_start(xt_sb[:], aps["xt"][b, 128 * i:128 * (i + 1), :])
                nc.tensor.matmul(Ph[:64, :C], lhsT=ABT[:, i, :], rhs=xt_sb[:],
                                 start=(i == 0), stop=(i == NL - 1))
            hTb = small.tile([64, C], F32, tag="hT")
            nc.vector.tensor_scalar_mul(hTb[:], Ph[:64, :C], rsum[:])
            hT.append(hTb)

            # ---- h natural via PE transpose ----
            for k in range(KC):
                Pt = pS.tile([128, NW], F32, tag="pS")
                nc.tensor.transpose(Pt[:128, :64], hTb[:, 128 * k:128 * (k + 1)],
                                    idf[0:64, :])
                copy_out(eng_rr, hnat[k][:, S * b:S * (b + 1)], Pt[:128, :64], None)
                eng_rr += 1

        # ================= batched small chain: hz / gate / ho =================
        hz = []
        for mo in range(2 * C // 128):
            Pz = pS.tile([128, NW], F32, tag="pS")
            for k in range(KC):
                nc.tensor.matmul(Pz[:, :BC * S],
                                 lhsT=whz_sb[k][:, 128 * mo:128 * (mo + 1)],
                                 rhs=hnat[k][:], start=(k == 0), stop=(k == KC - 1))
            t = small.tile([128, BC * S], F32, tag=f"hz{mo}")
            if use_bias:
                nc.vector.tensor_scalar_add(t[:], Pz[:, :BC * S],
                                            bhz[:, mo:mo + 1])
            else:
                copy_out(eng_rr, t[:], Pz[:, :BC * S], None)
                eng_rr += 1
            hz.append(t)

        hg = []
        for j in range(KC):
            sg = small.tile([128, BC * S], F32, tag=f"sg{j}")
            nc.scalar.activation(sg[:], hz[KC + j][:], AF.Sigmoid)
            nc.vector.tensor_mul(sg[:], sg[:], hz[KC + j][:])  # silu(z)
            t = small.tile([128, BC * S], BF16, tag=f"hg{j}")
            # hg = (silu(z) + dskip) * h1
            nc.vector.scalar_tensor_tensor(t[:], sg[:], float(dskip_val),
                                           hz[j][:], ALU.add, ALU.mult)
            hg.append(t)

        for b in range(BC):
            Po = pS.tile([128, NW], F32, tag="pS")
            for k in range(KC):
                nc.tensor.matmul(Po[:64, :C], lhsT=hg[k][:, S * b:S * (b + 1)],
                                 rhs=wout_sb[k][:], start=(k == 0),
                                 stop=(k == KC - 1))
            if use_bias:
                nc.vector.tensor_add(hoT[b][:], Po[:64, :C], bout_rep[:])
            else:
                copy_out(eng_rr, hoT[b][:], Po[:64, :C], None)
                eng_rr += 1
            nc.vector.tensor_copy(hoTb[b][:], hoT[b][:])
            nc.sync.dma_start(aps["hot"][b, :, :], hoT[b][:])

        # ================= y = ho @ Cm =================
        for b in range(BC):
            for mc in range(KC):
                for n in range(nch):
                    n0 = n * NW
                    nw = min(NW, L2 - n0)
                    Py = pA.tile([128, NW], F32, tag="pA")
                    nc.tensor.matmul(Py[:, :nw],
                                     lhsT=hoTb[b][:, 128 * mc:128 * (mc + 1)],
                                     rhs=cm_tiles[b][:, n0:n0 + nw],
                                     start=True, stop=True)
                    ys = ystage.tile([128, NW], F32, tag="ys")
                    copy_out(eng_rr, ys[:, :nw], Py[:, :nw], None)
                    eng_rr += 1
                    nc.sync.dma_start(
                        aps["ypad"][b, 128 * mc:128 * (mc + 1), n0:n0 + nw],
                        ys[:, :nw])


def build_program(BC, C, S, H, W, use_bias, dskip_val, reps=1):
    """Build + compile the SPMD program for one core (replicated on 8)."""
    W2 = W + 2
    L2 = H * W2
    nc = bacc.Bacc("TRN2", target_bir_lowering=False, debug=False,
                   num_devices=N_CORES)
    aps = {}

    def inp(name, shape, dt):
        aps[name] = nc.dram_tensor(name, shape, dt, kind="ExternalInput").ap()

    def outp(name, shape, dt):
        aps[name] = nc.dram_tensor(name, shape, dt, kind="ExternalOutput").ap()

    inp("xb", [BC, C, L2], BF16)
    inp("xt", [BC, L2, C], BF16)
    inp("w1t", [C, 3 * S], BF16)
    inp("dg0", [9, 128, 128], BF16)
    inp("dg1", [9, 64, 64], BF16)
    inp("whzT", [C, 2 * C], BF16)
    inp("woutT", [C, C], BF16)
    inp("identF", [128, 64], F32)
    if use_bias:
        inp("b_bcdt", [3 * S, 1], F32)
        inp("b_dw", [3 * S, 1], F32)
        inp("b_hz", [2 * C, 1], F32)
        inp("bout_rep", [64, C], F32)
    outp("ypad", [BC, C, L2], F32)
    outp("hot", [BC, S, C], F32)

    with tile.TileContext(nc, trace_sim=False) as tc:
        if reps > 1:
            with tc.For_i(0, reps, 1):
                emit_core_program(tc, aps, (BC, C, S, H, W), use_bias, dskip_val)
        else:
            emit_core_program(tc, aps, (BC, C, S, H, W), use_bias, dskip_val)
    nc.compile()
    return nc


def prep_inputs(x, w_bcdt, b_bcdt, w_dw, b_dw, w_hz, b_hz, w_out, b_out,
                A, Dskip, H, W):
    """Host-side prep: pad W dim, transpose, fold conv weights. Returns
    (in_maps, meta)."""
    H, W = int(H), int(W)
    B, C, L = x.shape
    S = A.shape[0]
    W2 = W + 2
    L2 = H * W2
    BC = B // N_CORES

    x = np.asarray(x, np.float32).reshape(B, C, H, W)
    xp = np.zeros((B, C, H, W2), np.float32)
    xp[:, :, :, 1:W + 1] = x
    xp = xp.reshape(B, C, L2)
    xb = xp.astype(ml_dtypes.bfloat16)
    xt = np.ascontiguousarray(xp.transpose(0, 2, 1)).astype(ml_dtypes.bfloat16)

    w1t = np.ascontiguousarray(np.asarray(w_bcdt, np.float32).T).astype(
        ml_dtypes.bfloat16)                       # [C, 3S]
    w9 = np.asarray(w_dw, np.float32).reshape(3 * S, 9)   # taps row-major (ki,kj)
    taps = [(0, 0)] + [(di, dj) for di in (-1, 0, 1) for dj in (-1, 0, 1)
                       if not (di == 0 and dj == 0)]
    dg0 = np.zeros((9, 128, 128), np.float32)
    dg1 = np.zeros((9, 64, 64), np.float32)
    for ti, (di, dj) in enumerate(taps):
        tap_idx = (di + 1) * 3 + (dj + 1)
        dg0[ti][np.arange(128), np.arange(128)] = w9[0:128, tap_idx]
        dg1[ti][np.arange(64), np.arange(64)] = w9[128:192, tap_idx]
    dg0 = dg0.astype(ml_dtypes.bfloat16)
    dg1 = dg1.astype(ml_dtypes.bfloat16)

    whzT = np.ascontiguousarray(np.asarray(w_hz, np.float32).T).astype(
        ml_dtypes.bfloat16)                       # [C, 2C]
    woutT = np.ascontiguousarray(np.asarray(w_out, np.float32).T).astype(
        ml_dtypes.bfloat16)                       # [C, C]
    identF = np.zeros((128, 64), np.float32)
    identF[0:64] = np.eye(64, dtype=np.float32)
    identF[64:128] = np.eye(64, dtype=np.float32)

    use_bias = bool(np.any(b_bcdt) or np.any(b_dw) or np.any(b_hz)
                    or np.any(b_out))
    dskip_val = float(np.asarray(Dskip).reshape(-1)[0])

    in_maps = []
    for c in range(N_CORES):
        sl = slice(c * BC, (c + 1) * BC)
        m = {
            "xb": np.ascontiguousarray(xb[sl]),
            "xt": np.ascontiguousarray(xt[sl]),
            "w1t": w1t, "dg0": dg0, "dg1": dg1,
            "whzT": whzT, "woutT": woutT, "identF": identF,
        }
        if use_bias:
            m["b_bcdt"] = np.asarray(b_bcdt, np.float32).reshape(3 * S, 1)
            m["b_dw"] = np.asarray(b_dw, np.float32).reshape(3 * S, 1)
            m["b_hz"] = np.asarray(b_hz, np.float32).reshape(2 * C, 1)
            m["bout_rep"] = np.broadcast_to(
                np.asarray(b_out, np.float32), (64, C)).copy()
        in_maps.append(m)

    meta = dict(B=B, BC=BC, C=C, S=S, H=H, W=W, W2=W2, L2=L2,
                use_bias=use_bias, dskip_val=dskip_val)
    return in_maps, meta


_PROGRAM_CACHE = {}


def get_program(meta, reps=1):
    key = (meta["BC"], meta["C"], meta["S"], meta["H"], meta["W"],
           meta["use_bias"], meta["dskip_val"], reps)
    if key not in _PROGRAM_CACHE:
        _PROGRAM_CACHE[key] = build_program(
            meta["BC"], meta["C"], meta["S"], meta["H"], meta["W"],
            meta["use_bias"], meta["dskip_val"], reps=reps)
    return _PROGRAM_CACHE[key]


def gather_outputs(results, meta):
    B, BC, C, S, H, W, W2, L2 = (meta[k] for k in
                                 ("B", "BC", "C", "S", "H", "W", "W2", "L2"))
    y = np.empty((B, C, H, W), np.float32)
    ho = np.empty((B, C, S), np.float32)
    for c in range(N_CORES):
        ypad = results[c]["ypad"].reshape(BC, C, H, W2)
        y[c * BC:(c + 1) * BC] = ypad[:, :, :, 1:W + 1]
        ho[c * BC:(c + 1) * BC] = results[c]["hot"].transpose(0, 2, 1)
    return y, ho


def kernel(**inputs):
    in_maps, meta = prep_inputs(**inputs)
    nc = get_program(meta)
    res = run_bass_kernel_spmd(nc, in_maps, list(range(N_CORES)))
    return gather_outputs(res.results, meta)
